# revision 1
# baseline (speedup 1.0000x reference)
"""Trainium2 Bass kernel for a causal single-head attention layer.

reference:
    v = inp @ Wv + bv; k = inp @ Wk + bk; q = inp @ Wq + bq      # [B,T,H]
    W = softmax(causal_mask(k @ q^T / sqrt(C)))                  # [B,T,T]
    out = W @ v                                                  # [B,T,H]

B=512, T=256, C=384, H=64. Pure data parallel over 8 NeuronCores
(64 batches each); batches are processed in pairs so the q/k projection
matmuls run with a 512-wide moving operand.

Layout: scores are computed transposed (S^T[s,t], s on partitions) with
lhsT=q^T slices / rhs=k^T; exp(S^T) in that layout is directly the
stationary operand for the P@V matmul, and V is projected directly in
[s,h] layout (x^T chunks stationary), so no transposes anywhere — the
host pre-transposes inp to [B/2, C, 2, T] (1KB DMA rows). Softmax
normalization rides a ones-column appended to V so the P@V matmul also
emits row sums; one reciprocal + one broadcast multiply finish it.
The causal mask is applied by gpsimd affine_select after exp
(max-subtraction is skipped: |scores/sqrt(C)| < ~3 for this problem, so
exp cannot overflow; softmax is shift-invariant). A short warm-up matmul
burst at kernel start brings the PE out of its cold HAM clock state
while the first input DMA is in flight.
"""

import numpy as np
import ml_dtypes

import concourse.bass as bass
import concourse.bacc as bacc
import concourse.mybir as mybir
import concourse.tile as tile
from concourse.bass import broadcast_tensor_aps
from concourse.bass_utils import run_bass_kernel_spmd

N_CORES = 8
B, T, C, H = 512, 256, 384, 64
NB = B // N_CORES          # batches per core
NP = NB // 2               # batch pairs per core
KC = C // 128              # contraction chunks
SCALE = C ** (-0.5)
F32 = mybir.dt.float32
BF16 = mybir.dt.bfloat16
AF = mybir.ActivationFunctionType


def _bmul(nc, out, a, b):
    a2, b2 = broadcast_tensor_aps(a, b)
    nc.vector.tensor_tensor(out, a2, b2, op=mybir.AluOpType.mult)


def _badd(nc, out, a, b):
    a2, b2 = broadcast_tensor_aps(a, b)
    nc.vector.tensor_tensor(out, a2, b2, op=mybir.AluOpType.add)


def build_nc():
    nc = bacc.Bacc("TRN2", target_bir_lowering=False, debug=False)
    x_h = nc.declare_dram_parameter("x", [NP, C, 2, T], BF16, isOutput=False)
    wqk_h = nc.declare_dram_parameter("wqk", [C, 2 * H], BF16, isOutput=False)
    wv_h = nc.declare_dram_parameter("wv", [C, H], BF16, isOutput=False)
    bqk_h = nc.declare_dram_parameter("bqk", [128, 1], F32, isOutput=False)
    bvb_h = nc.declare_dram_parameter("bvb", [128, H], F32, isOutput=False)
    # out[g, u, p, j, h] = attention output for batch 2g+j, t = u*128+p
    out_h = nc.declare_dram_parameter("out", [NP, 2, 128, 2, H], F32, isOutput=True)

    with tile.TileContext(nc) as tc:
        # PE warm-up: dummy matmuls with no input dependencies so the HAM
        # clock gate reaches 8/8 while the first input DMA streams. The
        # warm-up SBUF tile lives in the long-lived const pool so the
        # constant DMAs don't inherit an address-reuse dependency on it.
        with (
            tc.tile_pool(name="const", bufs=1) as const,
            tc.tile_pool(name="xp", bufs=4) as xp,
            tc.tile_pool(name="qkp", bufs=3) as qkp,
            tc.tile_pool(name="exp", bufs=4) as expp,
            tc.tile_pool(name="vp", bufs=3) as vp,
            tc.tile_pool(name="op", bufs=3) as op,
        ):
            with tc.tile_pool(name="warm_ps", bufs=1, space="PSUM") as warm_ps:
                wsb = const.tile([128, 512], BF16, tag="wsb")
                nc.gpsimd.memset(wsb[:], 1.0)
                wps = warm_ps.tile([128, 512], F32, tag="wps")
                for _ in range(12):
                    nc.tensor.matmul(
                        wps[:], wsb[:, 0:128], wsb[:], start=True, stop=True
                    )
            ctx_ps = tc.tile_pool(name="ps_qk", bufs=3, space="PSUM")
            ps_qk = ctx_ps.__enter__()
            ctx_v = tc.tile_pool(name="ps_v", bufs=2, space="PSUM")
            ps_v = ctx_v.__enter__()
            ctx_att = tc.tile_pool(name="ps_att", bufs=3, space="PSUM")
            ps_att = ctx_att.__enter__()

            # first pair's input DMA goes ahead of the constant loads so the
            # projection matmuls can start as soon as the warm-up drains.
            xt0 = xp.tile([128, KC, 2, T], BF16, tag="xt", name="xt0")
            nc.sync.dma_start(
                xt0[:], x_h.ap()[0].rearrange("(k p) j t -> p k j t", p=128)
            )

            wqk_sb = const.tile([128, KC, 2 * H], BF16, tag="wqk")
            nc.sync.dma_start(wqk_sb[:], wqk_h.ap().rearrange("(k p) h -> p k h", p=128))
            wv_sb = const.tile([128, KC, H], BF16, tag="wv")
            nc.sync.dma_start(wv_sb[:], wv_h.ap().rearrange("(k p) h -> p k h", p=128))
            bqk_sb = const.tile([128, 1], F32, tag="bqk")
            nc.sync.dma_start(bqk_sb[:], bqk_h.ap())
            bvb_sb = const.tile([128, H], F32, tag="bvb")
            nc.sync.dma_start(bvb_sb[:], bvb_h.ap())

            for g in range(NP):
                # ---- load x^T for the pair: [c_part, k, j, t] -----------
                if g == 0:
                    xt = xt0
                else:
                    xt = xp.tile([128, KC, 2, T], BF16, tag="xt", name=f"xt{g}")
                    nc.sync.dma_start(
                        xt[:], x_h.ap()[g].rearrange("(k p) j t -> p k j t", p=128)
                    )

                # ---- fused q^T|k^T projection (both batches at once) ----
                qk_ps = ps_qk.tile([128, 2, T], F32, tag="qk", name=f"qkps{g}")
                for k in range(KC):
                    nc.tensor.matmul(
                        qk_ps[:], wqk_sb[:, k, :], xt[:, k],
                        start=(k == 0), stop=(k == KC - 1),
                    )
                qt = qkp.tile([128, 2, T], BF16, tag="qt", name=f"qt{g}")
                nc.scalar.activation(qt[:], qk_ps[:], AF.Identity, bias=bqk_sb[:])
                # k^T half re-based to partition 0 (only DMA can shift
                # partitions) so the score matmul operands share a base.
                kt = qkp.tile([H, 2, T], BF16, tag="kt", name=f"kt{g}")
                nc.sync.dma_start(kt[:], qt[64:128])

                # ---- v in [s, h] layout (x^T chunks stationary) ---------
                v_ps = ps_v.tile([128, 2, 2, H], F32, tag="v", name=f"vps{g}")
                for j in range(2):
                    for si in range(2):
                        for k in range(KC):
                            nc.tensor.matmul(
                                v_ps[:, j, si, :],
                                xt[:, k, j, si * 128:(si + 1) * 128],
                                wv_sb[:, k, :],
                                start=(k == 0), stop=(k == KC - 1),
                            )
                vo = vp.tile([128, 2, 2, H + 1], BF16, tag="vo", name=f"vo{g}")
                nc.gpsimd.memset(vo[:, :, :, H:H + 1], 1.0)
                _badd(nc, vo[:, :, :, 0:H], v_ps[:], bvb_sb[:][:, None, None, :])

                # ---- attention (per batch) ------------------------------
                exs = []
                for j in range(2):
                    st_ps = ps_att.tile([128, 384], F32, tag="att", name=f"st{g}_{j}")
                    nc.tensor.matmul(
                        st_ps[:, 0:T], qt[0:H, j, 0:128], kt[:, j, :],
                        start=True, stop=True,
                    )
                    nc.tensor.matmul(
                        st_ps[:, T:T + 128], qt[0:H, j, 128:T], kt[:, j, 128:T],
                        start=True, stop=True,
                    )
                    ex = expp.tile([128, 384], BF16, tag="ex", name=f"ex{g}_{j}")
                    nc.scalar.activation(ex[:], st_ps[:], AF.Exp, scale=SCALE)
                    nc.gpsimd.affine_select(
                        out=ex[:, 0:T], in_=ex[:, 0:T],
                        compare_op=mybir.AluOpType.is_ge, fill=0.0,
                        base=0, pattern=[[1, T]], channel_multiplier=-1,
                    )
                    nc.gpsimd.affine_select(
                        out=ex[:, T:T + 128], in_=ex[:, T:T + 128],
                        compare_op=mybir.AluOpType.is_ge, fill=0.0,
                        base=0, pattern=[[1, 128]], channel_multiplier=-1,
                    )
                    exs.append(ex)

                # ---- out = P @ [v | 1], layout [u, j] for packed store --
                ou_ps = ps_att.tile([128, 2, 2, H + 1], F32, tag="att", name=f"ou{g}")
                for j in range(2):
                    ex = exs[j]
                    nc.tensor.matmul(
                        ou_ps[:, 0, j, :], ex[:, 0:128], vo[:, j, 0, :],
                        start=True, stop=True,
                    )
                    nc.tensor.matmul(
                        ou_ps[:, 1, j, :], ex[:, 128:T], vo[:, j, 0, :],
                        start=True, stop=False,
                    )
                    nc.tensor.matmul(
                        ou_ps[:, 1, j, :], ex[:, T:T + 128], vo[:, j, 1, :],
                        start=False, stop=True,
                    )

                # ---- normalize + store (both batches at once) -----------
                rec = op.tile([128, 2, 2, 1], F32, tag="rec", name=f"rec{g}")
                nc.vector.reciprocal(rec[:], ou_ps[:, :, :, H:H + 1])
                ot = op.tile([128, 2, 2, H], F32, tag="ot", name=f"ot{g}")
                _bmul(nc, ot[:], ou_ps[:, :, :, 0:H], rec[:])
                nc.sync.dma_start(
                    out_h.ap()[g].rearrange("u p j h -> p u j h"), ot[:]
                )
            ctx_att.__exit__(None, None, None)
            ctx_v.__exit__(None, None, None)
            ctx_ps.__exit__(None, None, None)
    nc.compile()
    return nc


_NC_CACHE = None


def _get_nc():
    global _NC_CACHE
    if _NC_CACHE is None:
        _NC_CACHE = build_nc()
    return _NC_CACHE


def prep_in_maps(inp, Wv, bv, Wk, bk, Wq, bq):
    """Host-side shard + layout prep. Returns the 8 per-core input maps."""
    bf16 = ml_dtypes.bfloat16
    wqk_b = np.ascontiguousarray(
        np.concatenate(
            [np.asarray(Wq, np.float32), np.asarray(Wk, np.float32)], axis=1
        ).astype(bf16)
    )
    wv_b = np.ascontiguousarray(np.asarray(Wv, np.float32).astype(bf16))
    bqk_c = np.ascontiguousarray(
        np.concatenate(
            [np.asarray(bq, np.float32).reshape(H), np.asarray(bk, np.float32).reshape(H)]
        ).reshape(128, 1)
    )
    bvb = np.ascontiguousarray(
        np.tile(np.asarray(bv, np.float32).reshape(1, H), (128, 1))
    )
    inp = np.asarray(inp, np.float32)
    in_maps = []
    for c in range(N_CORES):
        shard = inp[c * NB:(c + 1) * NB]                    # [NB, T, C]
        x_t = np.ascontiguousarray(
            shard.reshape(NP, 2, T, C).transpose(0, 3, 1, 2).astype(bf16)
        )                                                    # [NP, C, 2, T]
        in_maps.append({
            "x": x_t, "wqk": wqk_b, "wv": wv_b, "bqk": bqk_c, "bvb": bvb,
        })
    return in_maps


def unpack_out(results):
    """results: list of per-core dicts -> full [B, T, H] float32 output."""
    outs = []
    for c in range(N_CORES):
        o = results[c]["out"]                  # [NP, 2, 128, 2, H]
        outs.append(o.transpose(0, 3, 1, 2, 4).reshape(NB, T, H))
    return np.concatenate(outs, axis=0)


def kernel(inp, Wv, bv, Wk, bk, Wq, bq):
    in_maps = prep_in_maps(inp, Wv, bv, Wk, bk, Wq, bq)
    nc = _get_nc()
    res = run_bass_kernel_spmd(nc, in_maps, core_ids=list(range(N_CORES)))
    return unpack_out(res.results)



# revision 4
# speedup vs baseline: 1.1499x; 1.1499x over previous
"""Trainium2 Bass kernel for a causal single-head attention layer.

reference:
    v = inp @ Wv + bv; k = inp @ Wk + bk; q = inp @ Wq + bq      # [B,T,H]
    W = softmax(causal_mask(k @ q^T / sqrt(C)))                  # [B,T,T]
    out = W @ v                                                  # [B,T,H]

B=512, T=256, C=384, H=64. Pure data parallel over 8 NeuronCores
(64 batches each); batches are processed in QUADS (4 at a time, 16
iterations per core) with a 3-stage software pipeline:

    iteration i issues:  A = projections(quad i)
                         B = scores+exp+mask(quad i-1)
                         C = P@V+normalize+store(quad i-2)

so every Tensor-queue instruction's inputs were produced a full
iteration earlier and the PE never stalls on the exp/mask cross-engine
latency (which also keeps the HAM clock gate at K=8/8).

Layout: scores are computed transposed (S^T[s,t], s on partitions) with
lhsT=q^T slices / rhs=k^T packed into two PSUM tiles per quad
(stA = t:0..256 for s-block 0, stB = t:128..256 for s-block 1); exp(S^T)
lands in one [128, 4, 384] bf16 tile which directly provides the
stationary chunks for the P@V matmuls. V is projected directly in [s,h]
layout (x^T chunks stationary). Softmax normalization rides a
ones-column appended to V so the P@V matmul also emits row sums; one
reciprocal + one broadcast multiply finish it. The causal mask (needed
only on the two diagonal 128x128 blocks) is one precomputed tile
applied by gpsimd multiplies after exp (max-subtraction is skipped:
|scores/sqrt(C)| < ~3 for this problem, so exp cannot overflow).
The output is stored as bf16 (cast to f32 on host).

Engine budget per quad (warm, ns): Tensor ~2900 (pacing), Vector ~2650
(qt/v evac + recip + normalize), Scalar ~2050 (exp), GpSimd ~1500
(masks), Sync ~2000 (3 batched DMAs).
"""

import numpy as np
import ml_dtypes

import concourse.bass as bass
import concourse.bacc as bacc
import concourse.mybir as mybir
import concourse.tile as tile
from concourse.bass import broadcast_tensor_aps
from concourse.bass_utils import run_bass_kernel_spmd

N_CORES = 8
B, T, C, H = 512, 256, 384, 64
NB = B // N_CORES          # batches per core
NQ = NB // 4               # batch quads per core
KC = C // 128              # contraction chunks
SCALE = C ** (-0.5)
F32 = mybir.dt.float32
BF16 = mybir.dt.bfloat16
AF = mybir.ActivationFunctionType


def _bmul(nc, out, a, b):
    a2, b2 = broadcast_tensor_aps(a, b)
    nc.vector.tensor_tensor(out, a2, b2, op=mybir.AluOpType.mult)


def _badd(nc, out, a, b):
    a2, b2 = broadcast_tensor_aps(a, b)
    nc.vector.tensor_tensor(out, a2, b2, op=mybir.AluOpType.add)


def _gmul(nc, out, a, b):
    a2, b2 = broadcast_tensor_aps(a, b)
    nc.gpsimd.tensor_tensor(out, a2, b2, op=mybir.AluOpType.mult)


def build_nc():
    nc = bacc.Bacc("TRN2", target_bir_lowering=False, debug=False)
    x_h = nc.declare_dram_parameter("x", [NQ, C, 4, T], BF16, isOutput=False)
    wqk_h = nc.declare_dram_parameter("wqk", [C, 2 * H], BF16, isOutput=False)
    wv_h = nc.declare_dram_parameter("wv", [C, H], BF16, isOutput=False)
    bqk_h = nc.declare_dram_parameter("bqk", [128, 1], F32, isOutput=False)
    bvb_h = nc.declare_dram_parameter("bvb", [128, H], F32, isOutput=False)
    # out[q, u, p, b, h] = attention output for batch 4q+b, t = u*128+p
    out_h = nc.declare_dram_parameter("out", [NQ, 2, 128, 4, H], BF16, isOutput=True)

    with tile.TileContext(nc) as tc:
        with (
            tc.tile_pool(name="const", bufs=1) as const,
            tc.tile_pool(name="xp", bufs=3) as xp,
            tc.tile_pool(name="qkp", bufs=2) as qkp,
            tc.tile_pool(name="exp", bufs=2) as expp,
            tc.tile_pool(name="vp", bufs=2) as vp,
            tc.tile_pool(name="op", bufs=2) as op,
        ):
            # PE warm-up: dummy matmuls with no input dependencies so the HAM
            # clock gate ramps while the first input DMA streams.
            with tc.tile_pool(name="warm_ps", bufs=1, space="PSUM") as warm_ps:
                wsb = const.tile([128, 512], BF16, tag="wsb")
                nc.gpsimd.memset(wsb[:], 1.0)
                wps = warm_ps.tile([128, 512], F32, tag="wps")
                for _ in range(12):
                    nc.tensor.matmul(
                        wps[:], wsb[:, 0:128], wsb[:], start=True, stop=True
                    )

            ctxs = []

            def psum_pool(name, bufs=1):
                ctx = tc.tile_pool(name=name, bufs=bufs, space="PSUM")
                ctxs.append(ctx)
                return ctx.__enter__()

            ps_qk = psum_pool("ps_qk")    # [128, 2, 512] f32 = 2 banks
            ps_v = psum_pool("ps_v")      # [128, 4, 2, 64] f32 = 1 bank
            ps_stA = psum_pool("ps_stA")  # [128, 4, 256] f32 = 2 banks
            ps_stB = psum_pool("ps_stB")  # [128, 4, 128] f32 = 1 bank
            ps_ou = psum_pool("ps_ou")    # [128, 2, 512] f32 = 2 banks

            # first quad's input DMA goes ahead of the constant loads so the
            # projection matmuls can start as soon as the warm-up drains.
            xts = {}
            xts[0] = xp.tile([128, KC, 4, T], BF16, tag="xt", name="xt0")
            nc.sync.dma_start(
                xts[0][:], x_h.ap()[0].rearrange("(k p) b t -> p k b t", p=128)
            )

            wqk_sb = const.tile([128, KC, 2 * H], BF16, tag="wqk")
            nc.sync.dma_start(wqk_sb[:], wqk_h.ap().rearrange("(k p) h -> p k h", p=128))
            wv_sb = const.tile([128, KC, H], BF16, tag="wv")
            nc.sync.dma_start(wv_sb[:], wv_h.ap().rearrange("(k p) h -> p k h", p=128))
            bqk_sb = const.tile([128, 1], F32, tag="bqk")
            nc.sync.dma_start(bqk_sb[:], bqk_h.ap())
            bvb_sb = const.tile([128, H], F32, tag="bvb")
            nc.sync.dma_start(bvb_sb[:], bvb_h.ap())

            # causal mask for the two diagonal 128x128 blocks: keep col >= row
            mask_sb = const.tile([128, 128], BF16, tag="mask")
            nc.gpsimd.memset(mask_sb[:], 1.0)
            nc.gpsimd.affine_select(
                out=mask_sb[:], in_=mask_sb[:],
                compare_op=mybir.AluOpType.is_ge, fill=0.0,
                base=0, pattern=[[1, 128]], channel_multiplier=-1,
            )

            xts[1] = xp.tile([128, KC, 4, T], BF16, tag="xt", name="xt1")
            nc.sync.dma_start(
                xts[1][:], x_h.ap()[1].rearrange("(k p) b t -> p k b t", p=128)
            )

            qts, kts, exs, vos = {}, {}, {}, {}

            for i in range(NQ + 2):
                # ---------------- stage A: projections for quad i ----------
                if i < NQ:
                    q = i
                    if q + 2 < NQ:
                        xts[q + 2] = xp.tile(
                            [128, KC, 4, T], BF16, tag="xt", name=f"xt{q + 2}"
                        )
                        nc.sync.dma_start(
                            xts[q + 2][:],
                            x_h.ap()[q + 2].rearrange("(k p) b t -> p k b t", p=128),
                        )
                    xt = xts[q]

                    # fused q^T|k^T projection, two 512-col groups of 2 batches
                    qk_ps = ps_qk.tile([128, 2, 512], F32, tag="qk", name=f"qk{q}")
                    for grp in range(2):
                        for k in range(KC):
                            nc.tensor.matmul(
                                qk_ps[:, grp, :],
                                wqk_sb[:, k, :],
                                xt[:, k, 2 * grp:2 * grp + 2, :],
                                start=(k == 0), stop=(k == KC - 1),
                            )
                    qt = qkp.tile([128, 4, T], BF16, tag="qt", name=f"qt{q}")
                    _badd(
                        nc,
                        qt[:].rearrange("p b t -> p (b t)"),
                        qk_ps[:].rearrange("p g n -> p (g n)"),
                        bqk_sb[:],
                    )
                    # k^T half re-based to partition 0 (only DMA can shift
                    # partitions) so the score matmul operands share a base.
                    kt = qkp.tile([H, 4, T], BF16, tag="kt", name=f"kt{q}")
                    nc.sync.dma_start(kt[:], qt[64:128])
                    qts[q], kts[q] = qt, kt

                    # v in [s, h] layout (x^T chunks stationary)
                    v_ps = ps_v.tile([128, 4, 2, H], F32, tag="v", name=f"v{q}")
                    for b in range(4):
                        for si in range(2):
                            for k in range(KC):
                                nc.tensor.matmul(
                                    v_ps[:, b, si, :],
                                    xt[:, k, b, si * 128:(si + 1) * 128],
                                    wv_sb[:, k, :],
                                    start=(k == 0), stop=(k == KC - 1),
                                )
                    vo = vp.tile([128, 4, 2, H + 1], BF16, tag="vo", name=f"vo{q}")
                    nc.gpsimd.memset(vo[:, :, :, H:H + 1], 1.0)
                    _badd(nc, vo[:, :, :, 0:H], v_ps[:], bvb_sb[:][:, None, None, :])
                    vos[q] = vo

                # ---------------- stage B: scores/softmax for quad i-1 -----
                if 0 <= i - 1 < NQ:
                    q = i - 1
                    qt, kt = qts[q], kts[q]
                    stA = ps_stA.tile([128, 4, 256], F32, tag="stA", name=f"stA{q}")
                    stB = ps_stB.tile([128, 4, 128], F32, tag="stB", name=f"stB{q}")
                    for b in range(4):
                        nc.tensor.matmul(
                            stA[:, b, :], qt[0:H, b, 0:128], kt[:, b, :],
                            start=True, stop=True,
                        )
                        nc.tensor.matmul(
                            stB[:, b, :], qt[0:H, b, 128:T], kt[:, b, 128:T],
                            start=True, stop=True,
                        )
                    ex = expp.tile([128, 4, 384], BF16, tag="ex", name=f"ex{q}")
                    nc.scalar.activation(ex[:, :, 0:256], stA[:], AF.Exp, scale=SCALE)
                    nc.scalar.activation(ex[:, :, 256:384], stB[:], AF.Exp, scale=SCALE)
                    # causal mask on the two diagonal blocks
                    _gmul(nc, ex[:, :, 0:128], ex[:, :, 0:128], mask_sb[:][:, None, :])
                    _gmul(
                        nc, ex[:, :, 256:384], ex[:, :, 256:384],
                        mask_sb[:][:, None, :],
                    )
                    exs[q] = ex
                    del qts[q], kts[q]

                # ---------------- stage C: P@V + normalize + store ---------
                if 0 <= i - 2 < NQ:
                    q = i - 2
                    ex, vo = exs[q], vos[q]
                    # ou[:, u, b, 0:65] = [out | rowsum] for t-block u
                    ou = ps_ou.tile([128, 2, 4, 128], F32, tag="ou", name=f"ou{q}")
                    for b in range(4):
                        o0 = ou[:, 0, b, 0:65]
                        o1 = ou[:, 1, b, 0:65]
                        nc.tensor.matmul(
                            o0, ex[:, b, 0:128], vo[:, b, 0, :],
                            start=True, stop=True,
                        )
                        nc.tensor.matmul(
                            o1, ex[:, b, 128:256], vo[:, b, 0, :],
                            start=True, stop=False,
                        )
                        nc.tensor.matmul(
                            o1, ex[:, b, 256:384], vo[:, b, 1, :],
                            start=False, stop=True,
                        )
                    rec = op.tile([128, 2, 4, 1], F32, tag="rec", name=f"rec{q}")
                    nc.vector.reciprocal(rec[:], ou[:, :, :, H:H + 1])
                    ot = op.tile([128, 2, 4, H], BF16, tag="ot", name=f"ot{q}")
                    _bmul(nc, ot[:], ou[:, :, :, 0:H], rec[:])
                    nc.sync.dma_start(
                        out_h.ap()[q].rearrange("u p b h -> p u b h"), ot[:]
                    )
                    del exs[q], vos[q]

            for ctx in reversed(ctxs):
                ctx.__exit__(None, None, None)
    nc.compile()
    return nc


_NC_CACHE = None


def _get_nc():
    global _NC_CACHE
    if _NC_CACHE is None:
        _NC_CACHE = build_nc()
    return _NC_CACHE


def prep_in_maps(inp, Wv, bv, Wk, bk, Wq, bq):
    """Host-side shard + layout prep. Returns the 8 per-core input maps."""
    bf16 = ml_dtypes.bfloat16
    wqk_b = np.ascontiguousarray(
        np.concatenate(
            [np.asarray(Wq, np.float32), np.asarray(Wk, np.float32)], axis=1
        ).astype(bf16)
    )
    wv_b = np.ascontiguousarray(np.asarray(Wv, np.float32).astype(bf16))
    bqk_c = np.ascontiguousarray(
        np.concatenate(
            [np.asarray(bq, np.float32).reshape(H), np.asarray(bk, np.float32).reshape(H)]
        ).reshape(128, 1)
    )
    bvb = np.ascontiguousarray(
        np.tile(np.asarray(bv, np.float32).reshape(1, H), (128, 1))
    )
    inp = np.asarray(inp, np.float32)
    in_maps = []
    for c in range(N_CORES):
        shard = inp[c * NB:(c + 1) * NB]                    # [NB, T, C]
        x_t = np.ascontiguousarray(
            shard.reshape(NQ, 4, T, C).transpose(0, 3, 1, 2).astype(bf16)
        )                                                    # [NQ, C, 4, T]
        in_maps.append({
            "x": x_t, "wqk": wqk_b, "wv": wv_b, "bqk": bqk_c, "bvb": bvb,
        })
    return in_maps


def unpack_out(results):
    """results: list of per-core dicts -> full [B, T, H] float32 output."""
    outs = []
    for c in range(N_CORES):
        o = np.asarray(results[c]["out"], dtype=np.float32)  # [NQ, 2, 128, 4, H]
        outs.append(o.transpose(0, 3, 1, 2, 4).reshape(NB, T, H))
    return np.concatenate(outs, axis=0)


def kernel(inp, Wv, bv, Wk, bk, Wq, bq):
    in_maps = prep_in_maps(inp, Wv, bv, Wk, bk, Wq, bq)
    nc = _get_nc()
    res = run_bass_kernel_spmd(nc, in_maps, core_ids=list(range(N_CORES)))
    return unpack_out(res.results)


# revision 9
# speedup vs baseline: 1.1937x; 1.0381x over previous
"""Trainium2 Bass kernel for a causal single-head attention layer.

reference:
    v = inp @ Wv + bv; k = inp @ Wk + bk; q = inp @ Wq + bq      # [B,T,H]
    W = softmax(causal_mask(k @ q^T / sqrt(C)))                  # [B,T,T]
    out = W @ v                                                  # [B,T,H]

B=512, T=256, C=384, H=64. Pure data parallel over 8 NeuronCores
(64 batches each); batches are processed in QUADS (4 at a time, 16
iterations per core) with a 3-stage software pipeline:

    iteration i issues:  A = projections(quad i)
                         B = scores+exp+mask(quad i-1)
                         C = P@V+normalize+store(quad i-2)

so every Tensor-queue instruction's inputs were produced a full
iteration earlier and the PE never stalls on the exp/mask cross-engine
latency (which also keeps the HAM clock gate at K=8/8).

Layout: scores are computed transposed (S^T[s,t], s on partitions) with
lhsT=q^T slices / rhs=k^T packed into two PSUM tiles per quad
(stA = t:0..256 for s-block 0, stB = t:128..256 for s-block 1); exp(S^T)
lands in one [128, 4, 384] bf16 tile which directly provides the
stationary chunks for the P@V matmuls. V is projected directly in [s,h]
layout (x^T chunks stationary). Softmax normalization rides a
ones-column appended to V so the P@V matmul also emits row sums; one
reciprocal + one broadcast multiply finish it. The causal mask (needed
only on the two diagonal 128x128 blocks) is one precomputed tile
applied by gpsimd multiplies after exp (max-subtraction is skipped:
|scores/sqrt(C)| < ~3 for this problem, so exp cannot overflow).
The output is stored as bf16 (cast to f32 on host).

Engine budget per quad (warm, ns): Tensor ~2900 (pacing), Vector ~2650
(qt/v evac + recip + normalize), Scalar ~2050 (exp), GpSimd ~1500
(masks), Sync ~2000 (3 batched DMAs).
"""

import numpy as np
import ml_dtypes

import concourse.bass as bass
import concourse.bacc as bacc
import concourse.mybir as mybir
import concourse.tile as tile
from concourse.bass import broadcast_tensor_aps
from concourse.bass_utils import run_bass_kernel_spmd

N_CORES = 8
B, T, C, H = 512, 256, 384, 64
NB = B // N_CORES          # batches per core
NQ = NB // 4               # batch quads per core
KC = C // 128              # contraction chunks
SCALE = C ** (-0.5)
F32 = mybir.dt.float32
BF16 = mybir.dt.bfloat16
AF = mybir.ActivationFunctionType


def _bmul(nc, out, a, b):
    a2, b2 = broadcast_tensor_aps(a, b)
    nc.vector.tensor_tensor(out, a2, b2, op=mybir.AluOpType.mult)


def _badd(nc, out, a, b):
    a2, b2 = broadcast_tensor_aps(a, b)
    nc.vector.tensor_tensor(out, a2, b2, op=mybir.AluOpType.add)


def _gmul(nc, out, a, b):
    a2, b2 = broadcast_tensor_aps(a, b)
    nc.gpsimd.tensor_tensor(out, a2, b2, op=mybir.AluOpType.mult)


def build_nc():
    nc = bacc.Bacc("TRN2", target_bir_lowering=False, debug=False)
    x_h = nc.declare_dram_parameter("x", [NQ, C, 4, T], BF16, isOutput=False)
    wqk_h = nc.declare_dram_parameter("wqk", [C, 2 * H], BF16, isOutput=False)
    wv_h = nc.declare_dram_parameter("wv", [C, H], BF16, isOutput=False)
    bqk_h = nc.declare_dram_parameter("bqk", [128, 1], F32, isOutput=False)
    bvb_h = nc.declare_dram_parameter("bvb", [128, H], F32, isOutput=False)
    # out[q, u, p, b, h] = attention output for batch 4q+b, t = u*128+p
    out_h = nc.declare_dram_parameter("out", [NQ, 2, 128, 4, H], BF16, isOutput=True)

    with tile.TileContext(nc) as tc:
        with (
            tc.tile_pool(name="const", bufs=1) as const,
            tc.tile_pool(name="xp", bufs=3) as xp,
            tc.tile_pool(name="qkp", bufs=3) as qkp,
            tc.tile_pool(name="exp", bufs=3) as expp,
            tc.tile_pool(name="vp", bufs=3) as vp,
            tc.tile_pool(name="op", bufs=3) as op,
        ):
            # PE warm-up: dummy matmuls with no input dependencies so the HAM
            # clock gate ramps while the first input DMA streams.
            with tc.tile_pool(name="warm_ps", bufs=1, space="PSUM") as warm_ps:
                wsb = const.tile([128, 512], BF16, tag="wsb")
                nc.gpsimd.memset(wsb[:], 1.0)
                wps = warm_ps.tile([128, 512], F32, tag="wps")
                for _ in range(12):
                    nc.tensor.matmul(
                        wps[:], wsb[:, 0:128], wsb[:], start=True, stop=True
                    )

            ctxs = []

            def psum_pool(name, bufs=1):
                ctx = tc.tile_pool(name=name, bufs=bufs, space="PSUM")
                ctxs.append(ctx)
                return ctx.__enter__()

            ps_qk = psum_pool("ps_qk")    # [128, 2, 512] f32 = 2 banks
            ps_v = psum_pool("ps_v")      # [128, 4, 2, 64] f32 = 1 bank
            ps_stA = psum_pool("ps_stA")  # [128, 4, 256] f32 = 2 banks
            ps_stB = psum_pool("ps_stB")  # [128, 4, 128] f32 = 1 bank
            ps_ou = psum_pool("ps_ou")    # [128, 2, 512] f32 = 2 banks

            # first quad's input DMA goes ahead of the constant loads so the
            # projection matmuls can start as soon as the warm-up drains.
            xts = {}
            xts[0] = xp.tile([128, KC, 4, T], BF16, tag="xt", name="xt0")
            nc.sync.dma_start(
                xts[0][:], x_h.ap()[0].rearrange("(k p) b t -> p k b t", p=128)
            )

            wqk_sb = const.tile([128, KC, 2 * H], BF16, tag="wqk")
            nc.sync.dma_start(wqk_sb[:], wqk_h.ap().rearrange("(k p) h -> p k h", p=128))
            wv_sb = const.tile([128, KC, H], BF16, tag="wv")
            nc.sync.dma_start(wv_sb[:], wv_h.ap().rearrange("(k p) h -> p k h", p=128))
            bqk_sb = const.tile([128, 1], F32, tag="bqk")
            nc.sync.dma_start(bqk_sb[:], bqk_h.ap())
            bvb_sb = const.tile([128, H], F32, tag="bvb")
            nc.sync.dma_start(bvb_sb[:], bvb_h.ap())

            xts[1] = xp.tile([128, KC, 4, T], BF16, tag="xt", name="xt1")
            nc.sync.dma_start(
                xts[1][:], x_h.ap()[1].rearrange("(k p) b t -> p k b t", p=128)
            )

            qts, kts, exs, vos = {}, {}, {}, {}

            for i in range(NQ + 2):
                # ---------------- stage A: projections for quad i ----------
                if i < NQ:
                    q = i
                    if q + 2 < NQ:
                        xts[q + 2] = xp.tile(
                            [128, KC, 4, T], BF16, tag="xt", name=f"xt{q + 2}"
                        )
                        nc.sync.dma_start(
                            xts[q + 2][:],
                            x_h.ap()[q + 2].rearrange("(k p) b t -> p k b t", p=128),
                        )
                    xt = xts[q]

                    # fused q^T|k^T projection, two 512-col groups of 2 batches
                    qk_ps = ps_qk.tile([128, 2, 512], F32, tag="qk", name=f"qk{q}")
                    for grp in range(2):
                        for k in range(KC):
                            nc.tensor.matmul(
                                qk_ps[:, grp, :],
                                wqk_sb[:, k, :],
                                xt[:, k, 2 * grp:2 * grp + 2, :],
                                start=(k == 0), stop=(k == KC - 1),
                            )
                    qt = qkp.tile([128, 4, T], BF16, tag="qt", name=f"qt{q}")
                    _badd(
                        nc,
                        qt[:].rearrange("p b t -> p (b t)"),
                        qk_ps[:].rearrange("p g n -> p (g n)"),
                        bqk_sb[:],
                    )
                    qts[q] = qt

                    # v in [s, h] layout (x^T chunks stationary)
                    v_ps = ps_v.tile([128, 4, 2, H], F32, tag="v", name=f"v{q}")
                    for b in range(4):
                        for si in range(2):
                            for k in range(KC):
                                nc.tensor.matmul(
                                    v_ps[:, b, si, :],
                                    xt[:, k, b, si * 128:(si + 1) * 128],
                                    wv_sb[:, k, :],
                                    start=(k == 0), stop=(k == KC - 1),
                                )
                    vo = vp.tile([128, 4, 2, H + 1], BF16, tag="vo", name=f"vo{q}")
                    nc.gpsimd.memset(vo[:, :, :, H:H + 1], 1.0)
                    _badd(nc, vo[:, :, :, 0:H], v_ps[:], bvb_sb[:][:, None, None, :])
                    vos[q] = vo

                # ---------------- stage B: scores/softmax for quad i-1 -----
                if 0 <= i - 1 < NQ:
                    q = i - 1
                    qt, kt = qts[q], kts[q]
                    stA = ps_stA.tile([128, 4, 256], F32, tag="stA", name=f"stA{q}")
                    stB = ps_stB.tile([128, 4, 128], F32, tag="stB", name=f"stB{q}")
                    for b in range(4):
                        nc.tensor.matmul(
                            stA[:, b, :], qt[0:H, b, 0:128], kt[:, b, :],
                            start=True, stop=True,
                        )
                        nc.tensor.matmul(
                            stB[:, b, :], qt[0:H, b, 128:T], kt[:, b, 128:T],
                            start=True, stop=True,
                        )
                    ex = expp.tile([128, 4, 384], BF16, tag="ex", name=f"ex{q}")
                    nc.scalar.activation(ex[:, :, 0:256], stA[:], AF.Exp, scale=SCALE)
                    nc.scalar.activation(ex[:, :, 256:384], stB[:], AF.Exp, scale=SCALE)
                    # causal mask on the two diagonal 128x128 blocks:
                    # keep col >= row, zero the rest (b-independent)
                    nc.gpsimd.affine_select(
                        out=ex[:, :, 0:128], in_=ex[:, :, 0:128],
                        compare_op=mybir.AluOpType.is_ge, fill=0.0,
                        base=0, pattern=[[0, 4], [1, 128]], channel_multiplier=-1,
                    )
                    nc.gpsimd.affine_select(
                        out=ex[:, :, 256:384], in_=ex[:, :, 256:384],
                        compare_op=mybir.AluOpType.is_ge, fill=0.0,
                        base=0, pattern=[[0, 4], [1, 128]], channel_multiplier=-1,
                    )
                    exs[q] = ex
                    del qts[q], kts[q]

                # ---------------- stage C: P@V + normalize + store ---------
                if 0 <= i - 2 < NQ:
                    q = i - 2
                    ex, vo = exs[q], vos[q]
                    # ou[:, u, b, 0:65] = [out | rowsum] for t-block u
                    ou = ps_ou.tile([128, 2, 4, 128], F32, tag="ou", name=f"ou{q}")
                    for b in range(4):
                        o0 = ou[:, 0, b, 0:65]
                        o1 = ou[:, 1, b, 0:65]
                        nc.tensor.matmul(
                            o0, ex[:, b, 0:128], vo[:, b, 0, :],
                            start=True, stop=True,
                        )
                        nc.tensor.matmul(
                            o1, ex[:, b, 128:256], vo[:, b, 0, :],
                            start=True, stop=False,
                        )
                        nc.tensor.matmul(
                            o1, ex[:, b, 256:384], vo[:, b, 1, :],
                            start=False, stop=True,
                        )
                    rec = op.tile([128, 2, 4, 1], F32, tag="rec", name=f"rec{q}")
                    nc.vector.reciprocal(rec[:], ou[:, :, :, H:H + 1])
                    ot = op.tile([128, 2, 4, H], BF16, tag="ot", name=f"ot{q}")
                    _bmul(nc, ot[:], ou[:, :, :, 0:H], rec[:])
                    nc.sync.dma_start(
                        out_h.ap()[q].rearrange("u p b h -> p u b h"), ot[:]
                    )
                    del exs[q], vos[q]

                # k^T half of quad i re-based to partition 0 (only DMA can
                # shift partitions) so the score matmul operands share a
                # base. Issued last so its wait on the qt evacuation can't
                # head-block the store/prefetch DMAs on the Sync queue.
                if i < NQ:
                    q = i
                    kt = qkp.tile([H, 4, T], BF16, tag="kt", name=f"kt{q}")
                    nc.sync.dma_start(kt[:], qts[q][64:128])
                    kts[q] = kt

            for ctx in reversed(ctxs):
                ctx.__exit__(None, None, None)
    nc.compile()
    return nc


_NC_CACHE = None


def _get_nc():
    global _NC_CACHE
    if _NC_CACHE is None:
        _NC_CACHE = build_nc()
    return _NC_CACHE


def prep_in_maps(inp, Wv, bv, Wk, bk, Wq, bq):
    """Host-side shard + layout prep. Returns the 8 per-core input maps."""
    bf16 = ml_dtypes.bfloat16
    wqk_b = np.ascontiguousarray(
        np.concatenate(
            [np.asarray(Wq, np.float32), np.asarray(Wk, np.float32)], axis=1
        ).astype(bf16)
    )
    wv_b = np.ascontiguousarray(np.asarray(Wv, np.float32).astype(bf16))
    bqk_c = np.ascontiguousarray(
        np.concatenate(
            [np.asarray(bq, np.float32).reshape(H), np.asarray(bk, np.float32).reshape(H)]
        ).reshape(128, 1)
    )
    bvb = np.ascontiguousarray(
        np.tile(np.asarray(bv, np.float32).reshape(1, H), (128, 1))
    )
    inp = np.asarray(inp, np.float32)
    in_maps = []
    for c in range(N_CORES):
        shard = inp[c * NB:(c + 1) * NB]                    # [NB, T, C]
        x_t = np.ascontiguousarray(
            shard.reshape(NQ, 4, T, C).transpose(0, 3, 1, 2).astype(bf16)
        )                                                    # [NQ, C, 4, T]
        in_maps.append({
            "x": x_t, "wqk": wqk_b, "wv": wv_b, "bqk": bqk_c, "bvb": bvb,
        })
    return in_maps


def unpack_out(results):
    """results: list of per-core dicts -> full [B, T, H] float32 output."""
    outs = []
    for c in range(N_CORES):
        o = np.asarray(results[c]["out"], dtype=np.float32)  # [NQ, 2, 128, 4, H]
        outs.append(o.transpose(0, 3, 1, 2, 4).reshape(NB, T, H))
    return np.concatenate(outs, axis=0)


def kernel(inp, Wv, bv, Wk, bk, Wq, bq):
    in_maps = prep_in_maps(inp, Wv, bv, Wk, bk, Wq, bq)
    nc = _get_nc()
    res = run_bass_kernel_spmd(nc, in_maps, core_ids=list(range(N_CORES)))
    return unpack_out(res.results)


# revision 12
# speedup vs baseline: 1.2408x; 1.0394x over previous
"""Trainium2 Bass kernel for a causal single-head attention layer.

reference:
    v = inp @ Wv + bv; k = inp @ Wk + bk; q = inp @ Wq + bq      # [B,T,H]
    W = softmax(causal_mask(k @ q^T / sqrt(C)))                  # [B,T,T]
    out = W @ v                                                  # [B,T,H]

B=512, T=256, C=384, H=64. Pure data parallel over 8 NeuronCores
(64 batches each); batches are processed in QUADS (4 at a time, 16
iterations per core) with a 3-stage software pipeline:

    iteration i issues:  A = projections(quad i)
                         B = scores+exp+mask(quad i-1)
                         C = P@V+normalize+store(quad i-2)

so every Tensor-queue instruction's inputs were produced a full
iteration earlier and the PE never stalls on the exp/mask cross-engine
latency (which also keeps the HAM clock gate at K=8/8).

Layout: scores are computed transposed (S^T[s,t], s on partitions) with
lhsT=q^T slices / rhs=k^T packed into two PSUM tiles per quad
(stA = t:0..256 for s-block 0, stB = t:128..256 for s-block 1); exp(S^T)
lands in one [128, 4, 384] bf16 tile which directly provides the
stationary chunks for the P@V matmuls. V is projected directly in [s,h]
layout (x^T chunks stationary). Softmax normalization rides a
ones-column appended to V so the P@V matmul also emits row sums; one
reciprocal + one broadcast multiply finish it. The causal mask (needed
only on the two diagonal 128x128 blocks) is one precomputed tile
applied by gpsimd multiplies after exp (max-subtraction is skipped:
|scores/sqrt(C)| < ~3 for this problem, so exp cannot overflow).
The output is stored as bf16 (cast to f32 on host).

Engine budget per quad (warm, ns): Tensor ~2900 (pacing), Vector ~2650
(qt/v evac + recip + normalize), Scalar ~2050 (exp), GpSimd ~1500
(masks), Sync ~2000 (3 batched DMAs).
"""

import numpy as np
import ml_dtypes

import concourse.bass as bass
import concourse.bacc as bacc
import concourse.mybir as mybir
import concourse.tile as tile
from concourse.bass import broadcast_tensor_aps
from concourse.bass_utils import run_bass_kernel_spmd

N_CORES = 8
B, T, C, H = 512, 256, 384, 64
NB = B // N_CORES          # batches per core
NQ = NB // 4               # batch quads per core
KC = C // 128              # contraction chunks
SCALE = C ** (-0.5)
F32 = mybir.dt.float32
BF16 = mybir.dt.bfloat16
AF = mybir.ActivationFunctionType


def _bmul(nc, out, a, b):
    a2, b2 = broadcast_tensor_aps(a, b)
    nc.vector.tensor_tensor(out, a2, b2, op=mybir.AluOpType.mult)


def _badd(nc, out, a, b):
    a2, b2 = broadcast_tensor_aps(a, b)
    nc.vector.tensor_tensor(out, a2, b2, op=mybir.AluOpType.add)


def _gmul(nc, out, a, b):
    a2, b2 = broadcast_tensor_aps(a, b)
    nc.gpsimd.tensor_tensor(out, a2, b2, op=mybir.AluOpType.mult)


def build_nc():
    nc = bacc.Bacc("TRN2", target_bir_lowering=False, debug=False)
    x_h = nc.declare_dram_parameter("x", [NQ, C, 4, T], BF16, isOutput=False)
    wqk_h = nc.declare_dram_parameter("wqk", [C, 2 * H], BF16, isOutput=False)
    wv_h = nc.declare_dram_parameter("wv", [C, H], BF16, isOutput=False)
    bqk_h = nc.declare_dram_parameter("bqk", [128, 1], F32, isOutput=False)
    bvb_h = nc.declare_dram_parameter("bvb", [128, H], F32, isOutput=False)
    # out[q, u, p, b, h] = attention output for batch 4q+b, t = u*128+p
    out_h = nc.declare_dram_parameter("out", [NQ, 2, 128, 4, H], BF16, isOutput=True)

    with tile.TileContext(nc) as tc:
        with (
            tc.tile_pool(name="const", bufs=1) as const,
            tc.tile_pool(name="xp", bufs=3) as xp,
            tc.tile_pool(name="qkp", bufs=3) as qkp,
            tc.tile_pool(name="exp", bufs=3) as expp,
            tc.tile_pool(name="op", bufs=3) as op,
        ):
            # PE warm-up: dummy matmuls with no input dependencies so the HAM
            # clock gate ramps while the first input DMA streams.
            with tc.tile_pool(name="warm_ps", bufs=1, space="PSUM") as warm_ps:
                wsb = const.tile([128, 512], BF16, tag="wsb")
                nc.gpsimd.memset(wsb[:], 1.0)
                wps = warm_ps.tile([128, 512], F32, tag="wps")
                for _ in range(12):
                    nc.tensor.matmul(
                        wps[:], wsb[:, 0:128], wsb[:], start=True, stop=True
                    )

            ctxs = []

            def psum_pool(name, bufs=1):
                ctx = tc.tile_pool(name=name, bufs=bufs, space="PSUM")
                ctxs.append(ctx)
                return ctx.__enter__()

            ps_qk = psum_pool("ps_qk")    # [128, 2, 512] f32 = 2 banks
            ps_v = psum_pool("ps_v")      # [128, 4, 2, 64] f32 = 1 bank
            ps_stA = psum_pool("ps_stA")  # [128, 4, 256] f32 = 2 banks
            ps_stB = psum_pool("ps_stB")  # [128, 4, 128] f32 = 1 bank
            ps_ou = psum_pool("ps_ou")    # [128, 2, 512] f32 = 2 banks

            # first quad's input DMA goes ahead of the constant loads so the
            # projection matmuls can start as soon as the warm-up drains.
            xts = {}
            xts[0] = xp.tile([128, KC, 4, T], BF16, tag="xt", name="xt0")
            nc.sync.dma_start(
                xts[0][:], x_h.ap()[0].rearrange("(k p) b t -> p k b t", p=128)
            )

            wqk_sb = const.tile([128, KC, 2 * H], BF16, tag="wqk")
            nc.sync.dma_start(wqk_sb[:], wqk_h.ap().rearrange("(k p) h -> p k h", p=128))
            wv_sb = const.tile([128, KC, H], BF16, tag="wv")
            nc.sync.dma_start(wv_sb[:], wv_h.ap().rearrange("(k p) h -> p k h", p=128))
            bqk_sb = const.tile([128, 1], F32, tag="bqk")
            nc.sync.dma_start(bqk_sb[:], bqk_h.ap())
            bvb_sb = const.tile([128, H], F32, tag="bvb")
            nc.sync.dma_start(bvb_sb[:], bvb_h.ap())

            xts[1] = xp.tile([128, KC, 4, T], BF16, tag="xt", name="xt1")
            nc.sync.dma_start(
                xts[1][:], x_h.ap()[1].rearrange("(k p) b t -> p k b t", p=128)
            )

            # v-with-ones-column tiles: persistent ring so the ones column is
            # written once here instead of a per-iteration memset (whose WAR
            # wait on the pool recycle was observed blocking the GpSimd queue
            # for microseconds, stalling the masks behind it).
            vo_ring = []
            for r in range(4):
                vt = const.tile([128, 4, 2, H + 1], BF16, tag=f"vor{r}")
                nc.gpsimd.memset(vt[:, :, :, H:H + 1], 1.0)
                vo_ring.append(vt)

            qts, kts, exs, vos = {}, {}, {}, {}

            for i in range(NQ + 2):
                # ---------------- stage A: projections for quad i ----------
                if i < NQ:
                    q = i
                    if q + 2 < NQ:
                        xts[q + 2] = xp.tile(
                            [128, KC, 4, T], BF16, tag="xt", name=f"xt{q + 2}"
                        )
                        nc.sync.dma_start(
                            xts[q + 2][:],
                            x_h.ap()[q + 2].rearrange("(k p) b t -> p k b t", p=128),
                        )
                    xt = xts[q]

                    # fused q^T|k^T projection, two 512-col groups of 2 batches
                    qk_ps = ps_qk.tile([128, 2, 512], F32, tag="qk", name=f"qk{q}")
                    for grp in range(2):
                        for k in range(KC):
                            nc.tensor.matmul(
                                qk_ps[:, grp, :],
                                wqk_sb[:, k, :],
                                xt[:, k, 2 * grp:2 * grp + 2, :],
                                start=(k == 0), stop=(k == KC - 1),
                            )
                    qt = qkp.tile([128, 4, T], BF16, tag="qt", name=f"qt{q}")
                    _badd(
                        nc,
                        qt[:].rearrange("p b t -> p (b t)"),
                        qk_ps[:].rearrange("p g n -> p (g n)"),
                        bqk_sb[:],
                    )
                    qts[q] = qt

                    # v in [s, h] layout (x^T chunks stationary)
                    v_ps = ps_v.tile([128, 4, 2, H], F32, tag="v", name=f"v{q}")
                    for b in range(4):
                        for si in range(2):
                            for k in range(KC):
                                nc.tensor.matmul(
                                    v_ps[:, b, si, :],
                                    xt[:, k, b, si * 128:(si + 1) * 128],
                                    wv_sb[:, k, :],
                                    start=(k == 0), stop=(k == KC - 1),
                                )
                    vo = vo_ring[q % 4]
                    _badd(nc, vo[:, :, :, 0:H], v_ps[:], bvb_sb[:][:, None, None, :])
                    vos[q] = vo

                # ---------------- stage B: scores/softmax for quad i-1 -----
                if 0 <= i - 1 < NQ:
                    q = i - 1
                    qt, kt = qts[q], kts[q]
                    stA = ps_stA.tile([128, 4, 256], F32, tag="stA", name=f"stA{q}")
                    stB = ps_stB.tile([128, 4, 128], F32, tag="stB", name=f"stB{q}")
                    for b in range(4):
                        nc.tensor.matmul(
                            stA[:, b, :], qt[0:H, b, 0:128], kt[:, b, :],
                            start=True, stop=True,
                        )
                        nc.tensor.matmul(
                            stB[:, b, :], qt[0:H, b, 128:T], kt[:, b, 128:T],
                            start=True, stop=True,
                        )
                    ex = expp.tile([128, 4, 384], BF16, tag="ex", name=f"ex{q}")
                    nc.scalar.activation(ex[:, :, 0:256], stA[:], AF.Exp, scale=SCALE)
                    nc.scalar.activation(ex[:, :, 256:384], stB[:], AF.Exp, scale=SCALE)
                    # causal mask on the two diagonal 128x128 blocks:
                    # keep col >= row, zero the rest (b-independent)
                    nc.gpsimd.affine_select(
                        out=ex[:, :, 0:128], in_=ex[:, :, 0:128],
                        compare_op=mybir.AluOpType.is_ge, fill=0.0,
                        base=0, pattern=[[0, 4], [1, 128]], channel_multiplier=-1,
                    )
                    nc.gpsimd.affine_select(
                        out=ex[:, :, 256:384], in_=ex[:, :, 256:384],
                        compare_op=mybir.AluOpType.is_ge, fill=0.0,
                        base=0, pattern=[[0, 4], [1, 128]], channel_multiplier=-1,
                    )
                    exs[q] = ex
                    del qts[q], kts[q]

                # ---------------- stage C: P@V + normalize + store ---------
                if 0 <= i - 2 < NQ:
                    q = i - 2
                    ex, vo = exs[q], vos[q]
                    # ou[:, u, b, 0:65] = [out | rowsum] for t-block u
                    ou = ps_ou.tile([128, 2, 4, 128], F32, tag="ou", name=f"ou{q}")
                    for b in range(4):
                        o0 = ou[:, 0, b, 0:65]
                        o1 = ou[:, 1, b, 0:65]
                        nc.tensor.matmul(
                            o0, ex[:, b, 0:128], vo[:, b, 0, :],
                            start=True, stop=True,
                        )
                        nc.tensor.matmul(
                            o1, ex[:, b, 128:256], vo[:, b, 0, :],
                            start=True, stop=False,
                        )
                        nc.tensor.matmul(
                            o1, ex[:, b, 256:384], vo[:, b, 1, :],
                            start=False, stop=True,
                        )
                    rec = op.tile([128, 2, 4, 1], F32, tag="rec", name=f"rec{q}")
                    nc.vector.reciprocal(rec[:], ou[:, :, :, H:H + 1])
                    ot = op.tile([128, 2, 4, H], BF16, tag="ot", name=f"ot{q}")
                    _bmul(nc, ot[:], ou[:, :, :, 0:H], rec[:])
                    nc.sync.dma_start(
                        out_h.ap()[q].rearrange("u p b h -> p u b h"), ot[:]
                    )
                    del exs[q], vos[q]

                # k^T half of quad i re-based to partition 0 (only DMA can
                # shift partitions) so the score matmul operands share a
                # base. Issued last so its wait on the qt evacuation can't
                # head-block the store/prefetch DMAs on the Sync queue.
                if i < NQ:
                    q = i
                    kt = qkp.tile([H, 4, T], BF16, tag="kt", name=f"kt{q}")
                    nc.sync.dma_start(kt[:], qts[q][64:128])
                    kts[q] = kt

            for ctx in reversed(ctxs):
                ctx.__exit__(None, None, None)
    nc.compile()
    return nc


_NC_CACHE = None


def _get_nc():
    global _NC_CACHE
    if _NC_CACHE is None:
        _NC_CACHE = build_nc()
    return _NC_CACHE


def prep_in_maps(inp, Wv, bv, Wk, bk, Wq, bq):
    """Host-side shard + layout prep. Returns the 8 per-core input maps."""
    bf16 = ml_dtypes.bfloat16
    wqk_b = np.ascontiguousarray(
        np.concatenate(
            [np.asarray(Wq, np.float32), np.asarray(Wk, np.float32)], axis=1
        ).astype(bf16)
    )
    wv_b = np.ascontiguousarray(np.asarray(Wv, np.float32).astype(bf16))
    bqk_c = np.ascontiguousarray(
        np.concatenate(
            [np.asarray(bq, np.float32).reshape(H), np.asarray(bk, np.float32).reshape(H)]
        ).reshape(128, 1)
    )
    bvb = np.ascontiguousarray(
        np.tile(np.asarray(bv, np.float32).reshape(1, H), (128, 1))
    )
    inp = np.asarray(inp, np.float32)
    in_maps = []
    for c in range(N_CORES):
        shard = inp[c * NB:(c + 1) * NB]                    # [NB, T, C]
        x_t = np.ascontiguousarray(
            shard.reshape(NQ, 4, T, C).transpose(0, 3, 1, 2).astype(bf16)
        )                                                    # [NQ, C, 4, T]
        in_maps.append({
            "x": x_t, "wqk": wqk_b, "wv": wv_b, "bqk": bqk_c, "bvb": bvb,
        })
    return in_maps


def unpack_out(results):
    """results: list of per-core dicts -> full [B, T, H] float32 output."""
    outs = []
    for c in range(N_CORES):
        o = np.asarray(results[c]["out"], dtype=np.float32)  # [NQ, 2, 128, 4, H]
        outs.append(o.transpose(0, 3, 1, 2, 4).reshape(NB, T, H))
    return np.concatenate(outs, axis=0)


def kernel(inp, Wv, bv, Wk, bk, Wq, bq):
    in_maps = prep_in_maps(inp, Wv, bv, Wk, bk, Wq, bq)
    nc = _get_nc()
    res = run_bass_kernel_spmd(nc, in_maps, core_ids=list(range(N_CORES)))
    return unpack_out(res.results)
